# revision 31
# baseline (speedup 1.0000x reference)
"""Trainium2 Bass kernel for nn_CausalAttention (B=2, S=2048, D=1024, H=16).

Sharding: tensor-parallel over heads (4 groups of 4 heads) x data-parallel
over batch (2), on 8 NeuronCores. Core c handles batch b = c // 4 and head
group g = c % 4 (heads 4g..4g+3, i.e. d_model columns 256g..256g+256).

Each core computes, fully on-device in bf16 (f32 PSUM accumulation):
  Q^T, K^T (d_out on partitions) and V (s on partitions, ones column
  appended) for its head slice, projected incrementally per q-chunk so
  projection matmuls overlap the (exp-latency-bound) attention pipeline;
  transposed score tiles (k on partitions, q free) trimmed to the causal
  region; P^T = exp(S^T / 8) with causal masking on GPSIMD (no max
  subtraction -- scores are O(1) here); unnormalized attention out^T[dh, q]
  via V-stationary matmuls where the ones column yields the softmax
  denominator for free; normalization by the broadcast fast-approx
  reciprocal of the denominator row; then a partial out = attn @ Wo_slice,
  interleaved per q-chunk.

Host-side glue (sharding/gather): x is pre-transposed per batch, weights are
pre-sliced and cast to bf16; the 4 partial outputs per batch are summed and
bo + bv @ Wo (the V-bias contribution, exact since softmax rows sum to 1)
is added.
"""

import sys

for _p in ("/opt/trn_rl_repo",):
    if _p not in sys.path:
        sys.path.append(_p)

import ml_dtypes
import numpy as np

import concourse.bass as bass
import concourse.mybir as mybir
import concourse.tile as tile
from concourse import bacc
from concourse.bass import ds, ts
from concourse.bass_utils import run_bass_kernel_spmd

B, S, D, H, DH = 2, 2048, 1024, 16, 64
N_CORES = 8
HPC = 4  # heads per core
DSL = HPC * DH  # 256, d_model slice per core
BF16 = mybir.dt.bfloat16
F32 = mybir.dt.float32

QC = 512  # q chunk for score tiles
KT = 128  # k tile (score-tile partition dim)
NQT = S // 128  # 16 q tiles of 128
NQC = S // QC  # 4 q chunks
NKC = D // 128  # 8 contraction chunks for projections
JPQ = QC // KT  # 4 k-tiles (and q-subtiles) per q chunk


def build_nc():
    nc = bacc.Bacc(
        "TRN2",
        target_bir_lowering=False,
        debug=False,
        enable_asserts=False,
        num_devices=N_CORES,
    )
    xT_ext = nc.dram_tensor("xT", [D, S], BF16, kind="ExternalInput")
    wq_ext = nc.dram_tensor("wq", [D, DSL], BF16, kind="ExternalInput")
    wk_ext = nc.dram_tensor("wk", [D, DSL], BF16, kind="ExternalInput")
    wv_ext = nc.dram_tensor("wv", [D, DSL], BF16, kind="ExternalInput")
    wo_ext = nc.dram_tensor("wo", [DSL, D], BF16, kind="ExternalInput")
    bq_ext = nc.dram_tensor("bq", [DSL], F32, kind="ExternalInput")
    bk_ext = nc.dram_tensor("bk", [DSL], F32, kind="ExternalInput")
    out_ext = nc.dram_tensor("out", [S, D], F32, kind="ExternalOutput")

    with tile.TileContext(nc) as tc:
        with (
            tc.tile_pool(name="consts", bufs=1) as consts,
            tc.tile_pool(name="weights", bufs=1) as weights,
            tc.tile_pool(name="xt", bufs=1) as xt_pool,
            tc.tile_pool(name="qkv", bufs=1) as qkv_pool,
            tc.tile_pool(name="pt", bufs=20) as pt_pool,
            tc.tile_pool(name="norm", bufs=8) as norm_pool,
            tc.tile_pool(name="out_sb", bufs=3) as out_pool,
            # PSUM: 3 x 2-bank (scores/proj) + 2 x 1-bank (P@V, out-proj)
            tc.tile_pool(name="s_psum", bufs=3, space="PSUM") as s_psum,
            tc.tile_pool(name="o_psum", bufs=2, space="PSUM") as o_psum,
        ):
            # --- constants ---
            bq_sb = consts.tile([128, 2], F32, name="bq_sb")
            bk_sb = consts.tile([128, 2], F32, name="bk_sb")
            nc.sync.dma_start(bq_sb[:], bq_ext.ap().rearrange("(c p) -> p c", p=128))
            nc.sync.dma_start(bk_sb[:], bk_ext.ap().rearrange("(c p) -> p c", p=128))

            # --- weight / xT loads ---
            # order matters: the first projection chain needs wq + xT chunks,
            # so those go first; wk/wv next; wo (needed last) at the end.
            wq_sb = weights.tile([128, NKC, DSL], BF16, name="wq_sb")
            wk_sb = weights.tile([128, NKC, DSL], BF16, name="wk_sb")
            wv_sb = weights.tile([128, NKC, DSL], BF16, name="wv_sb")
            wo_sb = weights.tile([128, 2, D], BF16, name="wo_sb")
            # scalar ring: wq/wk interleaved per chunk (the first Q/K
            # chains consume them chunk by chunk), then wv, then wo.
            for kc in range(NKC):
                nc.scalar.dma_start(wq_sb[:, kc, :], wq_ext.ap()[ts(kc, 128), :])
                nc.scalar.dma_start(wk_sb[:, kc, :], wk_ext.ap()[ts(kc, 128), :])
            nc.scalar.dma_start(
                wv_sb[:], wv_ext.ap().rearrange("(c p) n -> p c n", p=128)
            )
            nc.scalar.dma_start(
                wo_sb[:], wo_ext.ap().rearrange("(c p) n -> p c n", p=128)
            )
            # sync ring: xT column slices, first q-chunk first
            xt_sb = [[None] * NQC for _ in range(NKC)]
            for sc in range(NQC):
                for kc in range(NKC):
                    t = xt_pool.tile([128, QC], BF16, name=f"xt{kc}_{sc}")
                    nc.sync.dma_start(t[:], xT_ext.ap()[ts(kc, 128), ts(sc, QC)])
                    xt_sb[kc][sc] = t

            # --- projection / attention tiles ---
            qt_sb = [qkv_pool.tile([128, S], BF16, name=f"qt{c}") for c in range(2)]
            kt_sb = [qkv_pool.tile([128, S], BF16, name=f"kt{c}") for c in range(2)]
            v_sb = [
                qkv_pool.tile([128, HPC, DH + 1], BF16, name=f"v{st}")
                for st in range(NQT)
            ]
            attnT_sb = [qkv_pool.tile([128, S], BF16, name=f"att{c}") for c in range(2)]

            def proj_chunks(sc):
                """Emit-callbacks projecting Q^T/K^T columns and V s-tiles of
                q-chunk sc, one 8-matmul chain each."""

                def qk(dst, w_sb, b_sb, ci):
                    def emit():
                        ps = s_psum.tile([128, 2 * QC], F32, name="sc")
                        for kc in range(NKC):
                            nc.tensor.matmul(
                                ps[:, 0:QC],
                                w_sb[:, kc, ts(ci, 128)],
                                xt_sb[kc][sc][:],
                                start=(kc == 0),
                                stop=(kc == NKC - 1),
                            )
                        nc.vector.tensor_scalar_add(
                            out=dst[ci][:, ts(sc, QC)],
                            in0=ps[:, 0:QC],
                            scalar1=b_sb[:, ds(ci, 1)],
                        )

                    return emit

                def vproj(st):
                    def emit():
                        ps = s_psum.tile([128, 2 * QC], F32, name="sc")
                        for kc in range(NKC):
                            nc.tensor.matmul(
                                ps[:, 0:DSL],
                                xt_sb[kc][st // JPQ][:, ts(st % JPQ, 128)],
                                wv_sb[:, kc, :],
                                start=(kc == 0),
                                stop=(kc == NKC - 1),
                            )
                        nc.vector.tensor_copy(
                            v_sb[st][:, :, 0:DH],
                            ps[:, 0:DSL].rearrange("p (h d) -> p h d", h=HPC),
                        )
                        nc.gpsimd.memset(v_sb[st][:, :, DH : DH + 1], 1.0)

                    return emit

                chains = []
                for ci in range(2):
                    chains.append(qk(qt_sb, wq_sb, bq_sb, ci))
                    chains.append(qk(kt_sb, wk_sb, bk_sb, ci))
                for st in range(JPQ * sc, JPQ * (sc + 1)):
                    chains.append(vproj(st))
                return chains

            def tile_layout(qc):
                n_kt = (qc + 1) * JPQ
                width = [QC - KT * max(0, kt - qc * JPQ) for kt in range(n_kt)]
                off = [0 if kt % 2 == 0 else width[kt - 1] for kt in range(n_kt)]
                return n_kt, width, off

            def scores_group(qc, h, pt_g, g):
                ci, po = divmod(h, 2)
                po *= 64
                n_kt, width, off = tile_layout(qc)
                if True:
                    used = width[2 * g] + width[2 * g + 1]
                    ps = s_psum.tile([128, 2 * QC], F32, name="sc")
                    pt = pt_pool.tile([128, 2 * QC], BF16, name="pt")
                    for t in range(2):
                        kt = g * 2 + t
                        qoff = qc * QC + (QC - width[kt])
                        nc.tensor.matmul(
                            ps[:, ds(off[kt], width[kt])],
                            kt_sb[ci][po : po + 64, ts(kt, KT)],
                            qt_sb[ci][po : po + 64, ds(qoff, width[kt])],
                            start=True,
                            stop=True,
                        )
                    nc.scalar.activation(
                        pt[:, 0:used],
                        ps[:, 0:used],
                        mybir.ActivationFunctionType.Exp,
                        scale=0.125,
                    )
                    for t in range(2):
                        kt = g * 2 + t
                        if width[kt] < QC or kt == qc * JPQ:
                            # causal mask: zero where p > f (gpsimd, off the
                            # DVE critical path)
                            nc.gpsimd.affine_select(
                                out=pt[:, ds(off[kt], width[kt])],
                                in_=pt[:, ds(off[kt], width[kt])],
                                compare_op=mybir.AluOpType.is_ge,
                                fill=0.0,
                                base=0,
                                pattern=[[1, width[kt]]],
                                channel_multiplier=-1,
                            )
                    pt_g.append(pt)

            def scores(qc, h, pt_g):
                n_kt, _, _ = tile_layout(qc)
                for g in range(n_kt // 2):
                    scores_group(qc, h, pt_g, g)

            def pv_chain(qc, h, pt_g, po_, kts):
                ci, po = divmod(h, 2)
                n_kt, width, off = tile_layout(qc)
                for kt in kts:
                    g, t = divmod(kt, 2)
                    nc.tensor.matmul(
                        po_[0 : DH + 1, ds(QC - width[kt], width[kt])],
                        v_sb[kt][:, h, :],
                        pt_g[g][:, ds(off[kt], width[kt])],
                        start=(kt == 0),
                        stop=(kt == n_kt - 1),
                    )

            def pv_norm(qc, h, pt_g, po_=None, skip_chain=False):
                ci, po = divmod(h, 2)
                po *= 64
                n_kt, width, off = tile_layout(qc)
                # unnormalized out^T (+denominator row) for this (h, qc)
                if po_ is None:
                    po_ = o_psum.tile([128, QC], F32, name="ov")
                if not skip_chain:
                    pv_chain(qc, h, pt_g, po_, range(n_kt))
                # normalize: attnT[h rows, qc cols] = out^T * (1/denom)
                den = norm_pool.tile([64, QC], F32, name="den")
                row = norm_pool.tile([1, QC], F32, name="row")
                nc.vector.tensor_copy(row[:], po_[DH : DH + 1, :])
                nc.vector.reciprocal_approx_fast(den[0:1, :], row[:])
                nc.gpsimd.partition_broadcast(den[:], den[0:1, :])
                nc.vector.tensor_mul(
                    attnT_sb[ci][po : po + 64, ts(qc, QC)],
                    po_[0:DH, :],
                    den[:],
                )

            def out_proj_tile(qc, j):
                qt = qc * JPQ + j
                o_sb = out_pool.tile([128, D], F32, name="osb")
                for ncol in range(2):
                    pu = o_psum.tile([128, QC], F32, name="ov")
                    for ci in range(2):
                        nc.tensor.matmul(
                            pu[:],
                            attnT_sb[ci][:, ts(qt, 128)],
                            wo_sb[:, ci, ts(ncol, 512)],
                            start=(ci == 0),
                            stop=(ci == 1),
                        )
                    nc.vector.tensor_copy(o_sb[:, ts(ncol, 512)], pu[:])
                    nc.sync.dma_start(
                        out_ext.ap()[ts(qt, 128), ts(ncol, 512)],
                        o_sb[:, ts(ncol, 512)],
                    )

            # Engines execute strictly in program order, so PE bubbles in the
            # exp-bound attention pipeline must be filled by interleaving
            # independent matmul work (projection chains for q-chunk qc+1 and
            # the previous chunk's output projection) at emission granularity.
            # q-chunk 0 interleaves its own projections: scores for heads in
            # chunk half ci only need the Q/K chains of that half.
            c0 = proj_chunks(0)  # [Q0, K0, Q1, K1, V0..V3]
            pt_g0 = [[] for _ in range(HPC)]
            c0[0]()
            c0[1]()
            scores(0, 0, pt_g0[0])
            c0[2]()
            c0[3]()
            scores(0, 1, pt_g0[1])
            for emit in c0[4:]:
                emit()
            for qc in range(NQC):
                filler = proj_chunks(qc + 1) if qc + 1 < NQC else []
                fi = 0

                def fill(n):
                    nonlocal fi
                    for _ in range(n):
                        if fi < len(filler):
                            filler[fi]()
                            fi += 1

                def oprev(j):
                    if qc > 0:
                        out_proj_tile(qc - 1, j)

                pt_gs = pt_g0 if qc == 0 else [[] for _ in range(HPC)]
                # S0 S1 P0 S2 P1 S3 P2 P3 with proj filler and the previous
                # chunk's out-proj tiles in between
                if qc > 0:
                    scores(qc, 0, pt_gs[0])
                    fill(2)
                    oprev(0)
                    scores(qc, 1, pt_gs[1])
                    fill(1)
                else:
                    fill(3)
                pv_norm(qc, 0, pt_gs[0])
                fill(2)
                oprev(1)
                scores(qc, 2, pt_gs[2])
                fill(1)
                pv_norm(qc, 1, pt_gs[1])
                fill(2)
                oprev(2)
                if qc == NQC - 1:
                    # tail: pipeline head 3's P@V into its score groups so
                    # the final chain drains with the exps instead of after
                    n_kt = (qc + 1) * JPQ
                    po3 = o_psum.tile([128, QC], F32, name="ov")
                    for g in range(n_kt // 2):
                        scores_group(qc, 3, pt_gs[3], g)
                        if g >= 3:
                            pv_chain(qc, 3, pt_gs[3], po3, [2 * g - 6, 2 * g - 5])
                    pv_chain(qc, 3, pt_gs[3], po3, list(range(n_kt - 6, n_kt)))
                else:
                    scores(qc, 3, pt_gs[3])
                fill(1)
                pv_norm(qc, 2, pt_gs[2])
                fill(2)
                oprev(3)
                if qc == NQC - 1:
                    pv_norm(qc, 3, pt_gs[3], po_=po3, skip_chain=True)
                else:
                    pv_norm(qc, 3, pt_gs[3])
                fill(len(filler) - fi)
            for j in range(JPQ):
                out_proj_tile(NQC - 1, j)
            pass

    nc.compile()
    return nc


_NC_CACHE = None


def _get_nc():
    global _NC_CACHE
    if _NC_CACHE is None:
        _NC_CACHE = build_nc()
    return _NC_CACHE


def make_in_maps(x, Wq, bq, Wk, bk, Wv, bv, Wo, bo):
    bf = ml_dtypes.bfloat16
    in_maps = []
    for c in range(N_CORES):
        b, g = c // HPC, c % HPC
        lo, hi = g * DSL, (g + 1) * DSL
        in_maps.append(
            {
                "xT": np.ascontiguousarray(x[b].T).astype(bf),
                "wq": np.ascontiguousarray(Wq[:, lo:hi]).astype(bf),
                "wk": np.ascontiguousarray(Wk[:, lo:hi]).astype(bf),
                "wv": np.ascontiguousarray(Wv[:, lo:hi]).astype(bf),
                "wo": np.ascontiguousarray(Wo[lo:hi, :]).astype(bf),
                "bq": np.ascontiguousarray(bq[lo:hi]).astype(np.float32),
                "bk": np.ascontiguousarray(bk[lo:hi]).astype(np.float32),
            }
        )
    return in_maps


def gather_output(results, bv, Wo, bo):
    # softmax rows sum to 1, so the V-bias contributes bv @ Wo to every row
    corr = (np.asarray(bv, np.float64) @ np.asarray(Wo, np.float64)).astype(
        np.float32
    ) + np.asarray(bo, np.float32)
    out = np.empty((B, S, D), np.float32)
    for b in range(B):
        acc = np.zeros((S, D), np.float32)
        for g in range(HPC):
            acc += results[b * HPC + g]["out"].astype(np.float32)
        out[b] = acc + corr
    return out


def kernel(x, Wq, bq, Wk, bk, Wv, bv, Wo, bo, _trace=False):
    x = np.asarray(x, np.float32)
    nc = _get_nc()
    in_maps = make_in_maps(x, Wq, bq, Wk, bk, Wv, bv, Wo, bo)
    res = run_bass_kernel_spmd(nc, in_maps, list(range(N_CORES)), trace=_trace)
    out = gather_output(res.results, bv, Wo, bo)
    if _trace:
        return out, res
    return out
